# revision 12
# baseline (speedup 1.0000x reference)
"""Performer (FAVOR+ relu-kernel) block on 8 Trainium2 NeuronCores.

Data-parallel over batch: each of the 8 cores runs one batch element
(2048 tokens) through LN1 -> QKV -> linear attention -> wo+residual ->
LN2 -> FFN(gelu) -> gelu -> +residual, entirely in fp32.

Layout strategy per core:
  - Activations are token-major [t, e] for LN / elementwise (per-token
    stats live on partitions), and are PE-transposed to feature-major
    [e, t] chunks right before each matmul chain (contract dim must sit
    on partitions for both matmul operands).
  - q is produced feature-major directly (lhsT = wq); k, v token-major
    (lhsT = ln1T chunk, shared stationary operand).
  - kvs[h] = kp_h^T @ [v_h | 1] accumulates in PSUM across all token
    chunks (extra ones-column folds the ksum/denominator into the same
    matmul). num|den = qpT_h^T @ kvs_h in one matmul per (head, chunk).
  - FFN runs in two f-halves (w1/w2 fp32 don't fit SBUF together);
    partial out2 of half 0 spills to DRAM and is re-added in half 1.
LN gamma/beta are folded into the projection weights host-side (exact).
"""

import re

import numpy as np

E = 1024
H = 16
DH = 64
F = 4096
T = 2048
P = 128
EC = E // P  # 8 e-chunks
STAB = 1e-3
EPS_LN = 1e-6

_F32 = None  # set lazily (mybir import)


def _legalize_sync_waits(nc):
    """Split sync waits that exceed this walrus build's per-instruction cap.

    The pinned walrus rejects >1 sync wait on CTRL instructions (Drain,
    NoOp, EventSemaphore) and >2 on regular engine instructions, but
    Tile's wait-assignment emits as many as the dependency set needs.
    Move leading excess waits onto injected same-engine NoOps (1 wait
    each) placed immediately before the offending instruction.
    """
    import concourse.mybir as mybir
    import bass_rust

    ctrl_types = (mybir.InstDrain, mybir.InstNoOp, mybir.InstEventSemaphore)
    n_split = 0
    for f in nc.m.functions:
        for b in f.blocks:
            changed = False
            out = []
            for inst in b.instructions:
                si = inst.sync_info
                if si is not None:
                    waits = list(si.on_wait)
                    cap = 1
                    if len(waits) > cap:
                        excess, keep = waits[:-cap], waits[-cap:]
                        for w in excess:
                            nop = mybir.InstNoOp(
                                name=f"wsplit-{n_split}", ins=[], outs=[]
                            )
                            n_split += 1
                            nop.engine = inst.engine
                            nop.sync_info = bass_rust.SyncInfo(
                                on_wait=[w], on_update=[]
                            )
                            out.append(nop)
                        si.on_wait = keep
                        changed = True
                    ups = list(si.on_update)
                    assert len(ups) <= 2, (
                        f"{inst.name}: {len(ups)} sem updates; add update-split"
                    )
                out.append(inst)
            if changed:
                b.instructions = out


def build_program(T=T, n_cores=8):
    """Emit the per-core Bass/Tile program. Returns the Bass object."""
    from contextlib import ExitStack

    import concourse.bass as bass
    import concourse.mybir as mybir
    from concourse import tile

    f32 = mybir.dt.float32
    AF = mybir.ActivationFunctionType
    OP = mybir.AluOpType

    nc = bass.Bass("TRN2", target_bir_lowering=False, debug=False, num_devices=n_cores)

    x_d = nc.dram_tensor("x", [T, E], f32, kind="ExternalInput")
    wq_d = nc.dram_tensor("wq", [E, E], f32, kind="ExternalInput")
    wk_d = nc.dram_tensor("wk", [E, E], f32, kind="ExternalInput")
    wv_d = nc.dram_tensor("wv", [E, E], f32, kind="ExternalInput")
    wo_d = nc.dram_tensor("wo", [E, E], f32, kind="ExternalInput")
    w1_d = nc.dram_tensor("w1", [E, F], f32, kind="ExternalInput")
    w2_d = nc.dram_tensor("w2", [F, E], f32, kind="ExternalInput")
    b1_d = nc.dram_tensor("b1", [F], f32, kind="ExternalInput")
    id_d = nc.dram_tensor("ident", [P, P], f32, kind="ExternalInput")
    y_d = nc.dram_tensor("y", [T, E], f32, kind="ExternalOutput")

    # DRAM scratch (per-core local)
    qpT_d = nc.dram_tensor("qpT_s", [P, EC, T], f32)
    add1_d = nc.dram_tensor("add1_s", [T, E], f32)
    ln2T_d = nc.dram_tensor("ln2T_s", [P, EC, T], f32)
    o2p_d = nc.dram_tensor("o2p_s", [T, E], f32)

    NT = T // P            # token chunks of 128
    SP1 = 512              # phase 1/2 span
    NSP1 = T // SP1
    SP3 = 256              # phase 3 span
    NSP3 = T // SP3
    FH = F // 2            # f half
    FCH = FH // P          # 16 f-chunks per half

    STC = tile.TileContext

    def layer_norm_chunk(nc, pools, xc, out_tile):
        """token-major LN (no affine): out = (x - mean) * rsqrt(var+eps)."""
        st = pools["st"].tile([P, 2, 6], f32, tag="st")
        nc.vector.bn_stats(st[:, 0, :], xc[:, 0:512])
        nc.vector.bn_stats(st[:, 1, :], xc[:, 512:1024])
        ag = pools["st"].tile([P, 2], f32, tag="ag")
        nc.vector.bn_aggr(ag, st)
        vpe = pools["st"].tile([P, 1], f32, tag="vpe")
        nc.vector.tensor_scalar_add(vpe, ag[:, 1:2], EPS_LN)
        sd = pools["st"].tile([P, 1], f32, tag="sd")
        nc.scalar.activation(sd, vpe, AF.Sqrt)
        rs = pools["st"].tile([P, 1], f32, tag="rs")
        nc.vector.reciprocal(rs, sd)
        nm = pools["st"].tile([P, 1], f32, tag="nm")
        nc.vector.tensor_scalar(nm, ag[:, 0:1], rs[:, 0:1], -1.0, OP.mult, OP.mult)
        nc.scalar.activation(out_tile, xc, AF.Identity, bias=nm[:, 0:1], scale=rs[:, 0:1])

    def transpose_1024(nc, pools, src_ap_fn, dst_fn, ident):
        """Transpose a [128, 1024] token-major slab into 8 [128,128] blocks.

        src_ap_fn(j) -> [128, 128] SBUF AP ; dst_fn(g) -> [128, 4, 128] AP
        (dst for block group g covers blocks 4g..4g+3).
        """
        for g in range(2):
            tp = pools["tp"].tile([P, 4, P], f32, tag="tp")
            for jj in range(4):
                nc.tensor.transpose(tp[:, jj, :], src_ap_fn(4 * g + jj), ident)
            nc.vector.tensor_copy(dst_fn(g), tp)

    with STC(nc) as tc, ExitStack() as top:
        const_pool = top.enter_context(tc.tile_pool(name="const", bufs=1))
        ident = const_pool.tile([P, P], f32, tag="ident")
        nc.sync.dma_start(ident, id_d.ap())
        kvs_sb = const_pool.tile([P, H // 2, DH + 1], f32, tag="kvs_sb")

        # ================= PHASE 1: LN1, q/k/v, kvs =================
        with ExitStack() as ph1:
            wq_pool = ph1.enter_context(tc.tile_pool(name="wq", bufs=1))
            wq_sb = wq_pool.tile([P, EC, E], f32, tag="wq")
            wk_sb = wq_pool.tile([P, EC, E], f32, tag="wk")
            wv_sb = wq_pool.tile([P, EC, E], f32, tag="wv")
            for w_sb, w_d in ((wq_sb, wq_d), (wk_sb, wk_d), (wv_sb, wv_d)):
                w_r = w_d.ap().rearrange("(ec p) m -> p ec m", p=P)
                for ec in range(EC):
                    nc.sync.dma_start(w_sb[:, ec, :], w_r[:, ec, :])

            sb = {
                "st": ph1.enter_context(tc.tile_pool(name="st1", bufs=2)),
                "tp": ph1.enter_context(tc.tile_pool(name="tp1", bufs=2, space="PSUM")),
            }
            x_pool = ph1.enter_context(tc.tile_pool(name="x1", bufs=2))
            ln_pool = ph1.enter_context(tc.tile_pool(name="ln1", bufs=2))
            lnT_pool = ph1.enter_context(tc.tile_pool(name="lnT", bufs=2))
            qpT_pool = ph1.enter_context(tc.tile_pool(name="qpT", bufs=2))
            kvc_pool = ph1.enter_context(tc.tile_pool(name="kvc", bufs=2))
            q_ps = ph1.enter_context(tc.tile_pool(name="qps", bufs=1, space="PSUM"))
            kv_ps = ph1.enter_context(tc.tile_pool(name="kvps", bufs=3, space="PSUM"))
            kvs_ps_pool = ph1.enter_context(
                tc.tile_pool(name="kvsps", bufs=2, space="PSUM")
            )
            nc.vector.memset(kvs_sb, 0.0)

            for sp in range(NSP1):
                lnT_sp = lnT_pool.tile([P, EC, SP1], f32, tag="lnT")
                for tci in range(SP1 // P):
                    t0 = sp * SP1 + tci * P
                    xc = x_pool.tile([P, E], f32, tag="xc")
                    nc.sync.dma_start(xc, x_d.ap()[t0 : t0 + P, :])
                    lnc = ln_pool.tile([P, E], f32, tag="lnc")
                    layer_norm_chunk(nc, sb, xc, lnc)
                    transpose_1024(
                        nc,
                        sb,
                        lambda j: lnc[:, j * P : (j + 1) * P],
                        lambda g: lnT_sp[:, 4 * g : 4 * g + 4, tci * P : (tci + 1) * P],
                        ident,
                    )
                # q (feature-major)
                qpT_sp = qpT_pool.tile([P, EC, SP1], f32, tag="qpT")
                for mc in range(EC):
                    qp = q_ps.tile([P, SP1], f32, tag="q")
                    for ec in range(EC):
                        nc.tensor.matmul(
                            qp,
                            wq_sb[:, ec, mc * P : (mc + 1) * P],
                            lnT_sp[:, ec, :],
                            start=(ec == 0),
                            stop=(ec == EC - 1),
                        )
                    nc.vector.tensor_scalar(
                        qpT_sp[:, mc, :], qp, 0.0, STAB, OP.max, OP.add
                    )
                nc.sync.dma_start(qpT_d.ap()[:, :, sp * SP1 : (sp + 1) * SP1], qpT_sp)
                # k, v token-major + kvs accumulation
                for tci in range(SP1 // P):
                    gtc = sp * (SP1 // P) + tci
                    kc = kvc_pool.tile([P, E], f32, tag="kc")
                    vc = kvc_pool.tile([P, H, DH + 1], f32, tag="vc")
                    for half in range(2):
                        kps = kv_ps.tile([P, 512], f32, tag="kv")
                        for ec in range(EC):
                            nc.tensor.matmul(
                                kps,
                                lnT_sp[:, ec, tci * P : (tci + 1) * P],
                                wk_sb[:, ec, half * 512 : (half + 1) * 512],
                                start=(ec == 0),
                                stop=(ec == EC - 1),
                            )
                        nc.vector.tensor_scalar(
                            kc[:, half * 512 : (half + 1) * 512],
                            kps,
                            0.0,
                            STAB,
                            OP.max,
                            OP.add,
                        )
                    for half in range(2):
                        vps = kv_ps.tile([P, 512], f32, tag="kv")
                        for ec in range(EC):
                            nc.tensor.matmul(
                                vps,
                                lnT_sp[:, ec, tci * P : (tci + 1) * P],
                                wv_sb[:, ec, half * 512 : (half + 1) * 512],
                                start=(ec == 0),
                                stop=(ec == EC - 1),
                            )
                        nc.scalar.copy(
                            vc[:, half * 8 : (half + 1) * 8, 0:DH],
                            vps.rearrange("p (h d) -> p h d", h=8),
                        )
                    nc.vector.memset(vc[:, :, DH : DH + 1], 1.0)
                    for bank in range(2):
                        kvp = kvs_ps_pool.tile(
                            [P, 4, DH + 1], f32, tag="kvsp", name="kvp"
                        )
                        for hh in range(8):
                            h = bank * 8 + hh
                            r = h % 2
                            nc.tensor.matmul(
                                kvp[r * 64 : (r + 1) * 64, hh // 2, :],
                                kc[:, h * DH : (h + 1) * DH],
                                vc[:, h, :],
                                start=True,
                                stop=True,
                                tile_position=(0, r * 64),
                            )
                        nc.vector.tensor_tensor(
                            kvs_sb[:, bank * 4 : (bank + 1) * 4, :],
                            kvs_sb[:, bank * 4 : (bank + 1) * 4, :],
                            kvp,
                            OP.add,
                        )

        # ================= PHASE 2: attention out, add1, LN2 =================
        with ExitStack() as ph2:
            wo_pool = ph2.enter_context(tc.tile_pool(name="wo", bufs=1))
            wo_sb = wo_pool.tile([P, EC, E], f32, tag="wo")
            wo_r = wo_d.ap().rearrange("(ec p) m -> p ec m", p=P)
            for ec in range(EC):
                nc.sync.dma_start(wo_sb[:, ec, :], wo_r[:, ec, :])

            sb2 = {
                "st": ph2.enter_context(tc.tile_pool(name="st2", bufs=2)),
                "tp": ph2.enter_context(tc.tile_pool(name="tp2", bufs=2, space="PSUM")),
            }
            qpTs_pool = ph2.enter_context(tc.tile_pool(name="qpTs", bufs=2))
            att_pool = ph2.enter_context(tc.tile_pool(name="att", bufs=2))
            attT_pool = ph2.enter_context(tc.tile_pool(name="attT", bufs=2))
            x2_pool = ph2.enter_context(tc.tile_pool(name="x2", bufs=2))
            a1_pool = ph2.enter_context(tc.tile_pool(name="a1", bufs=2))
            ln2_pool = ph2.enter_context(tc.tile_pool(name="ln2", bufs=2))
            ln2T_pool = ph2.enter_context(tc.tile_pool(name="ln2T", bufs=2))
            num_ps = ph2.enter_context(tc.tile_pool(name="nps", bufs=2, space="PSUM"))
            wo_ps = ph2.enter_context(tc.tile_pool(name="wops", bufs=4, space="PSUM"))

            for sp in range(NSP1):
                qpT_sp = qpTs_pool.tile([P, EC, SP1], f32, tag="qpTs")
                nc.sync.dma_start(qpT_sp, qpT_d.ap()[:, :, sp * SP1 : (sp + 1) * SP1])
                att_sp = att_pool.tile([P, SP1 // P, H, DH], f32, tag="att")
                attT_sp = attT_pool.tile([P, EC, SP1], f32, tag="attT")
                ln2T_sp = ln2T_pool.tile([P, EC, SP1], f32, tag="ln2T")
                for tci in range(SP1 // P):
                    t0 = sp * SP1 + tci * P
                    for h in range(H):
                        r = h % 2
                        nps = num_ps.tile([P, DH + 1], f32, tag="num")
                        nc.tensor.matmul(
                            nps,
                            qpT_sp[r * 64 : (r + 1) * 64, h // 2, tci * P : (tci + 1) * P],
                            kvs_sb[r * 64 : (r + 1) * 64, h // 2, :],
                            start=True,
                            stop=True,
                        )
                        rc = sb2["st"].tile([P, 1], f32, tag="rc")
                        nc.vector.reciprocal(rc, nps[:, DH : DH + 1])
                        nc.vector.tensor_scalar_mul(
                            att_sp[:, tci, h, :], nps[:, 0:DH], rc[:, 0:1]
                        )
                    transpose_1024(
                        nc,
                        sb2,
                        lambda j: att_sp[:, tci, 2 * j : 2 * j + 2, :].rearrange(
                            "p h d -> p (h d)"
                        ),
                        lambda g: attT_sp[:, 4 * g : 4 * g + 4, tci * P : (tci + 1) * P],
                        ident,
                    )
                    xc2 = x2_pool.tile([P, E], f32, tag="xc2")
                    nc.sync.dma_start(xc2, x_d.ap()[t0 : t0 + P, :])
                    a1c = a1_pool.tile([P, E], f32, tag="a1c")
                    for half in range(2):
                        ops = wo_ps.tile([P, 512], f32, tag="wo")
                        for ec in range(EC):
                            nc.tensor.matmul(
                                ops,
                                attT_sp[:, ec, tci * P : (tci + 1) * P],
                                wo_sb[:, ec, half * 512 : (half + 1) * 512],
                                start=(ec == 0),
                                stop=(ec == EC - 1),
                            )
                        nc.vector.tensor_tensor(
                            a1c[:, half * 512 : (half + 1) * 512],
                            ops,
                            xc2[:, half * 512 : (half + 1) * 512],
                            OP.add,
                        )
                    nc.sync.dma_start(add1_d.ap()[t0 : t0 + P, :], a1c)
                    ln2c = ln2_pool.tile([P, E], f32, tag="ln2c")
                    layer_norm_chunk(nc, sb2, a1c, ln2c)
                    transpose_1024(
                        nc,
                        sb2,
                        lambda j: ln2c[:, j * P : (j + 1) * P],
                        lambda g: ln2T_sp[:, 4 * g : 4 * g + 4, tci * P : (tci + 1) * P],
                        ident,
                    )
                nc.sync.dma_start(ln2T_d.ap()[:, :, sp * SP1 : (sp + 1) * SP1], ln2T_sp)

        # ================= PHASE 3: FFN in two f-halves =================
        b1_sb_pool = top.enter_context(tc.tile_pool(name="b1p", bufs=1))
        b1_sb = b1_sb_pool.tile([P, F // P], f32, tag="b1")
        nc.sync.dma_start(b1_sb, b1_d.ap().rearrange("(c p) -> p c", p=P))
        w1_r = w1_d.ap().rearrange("(ec p) m -> p ec m", p=P)
        w2_r = w2_d.ap().rearrange("(fc p) m -> p fc m", p=P)

        for fh in range(2):
            with ExitStack() as ph3:
                wf_pool = ph3.enter_context(tc.tile_pool(name=f"wf{fh}", bufs=1))
                w1h = wf_pool.tile([P, EC, FH], f32, tag="w1h")
                w2h = wf_pool.tile([P, FCH, E], f32, tag="w2h")
                for ec in range(EC):
                    nc.sync.dma_start(
                        w1h[:, ec, :], w1_r[:, ec, fh * FH : (fh + 1) * FH]
                    )
                for fc in range(FCH):
                    nc.sync.dma_start(w2h[:, fc, :], w2_r[:, fh * FCH + fc, :])

                l2s_pool = ph3.enter_context(tc.tile_pool(name=f"l2s{fh}", bufs=2))
                hT_pool = ph3.enter_context(tc.tile_pool(name=f"hT{fh}", bufs=2))
                o_pool = ph3.enter_context(tc.tile_pool(name=f"o{fh}", bufs=1))
                os_pool = ph3.enter_context(tc.tile_pool(name=f"os{fh}", bufs=2))
                f1_ps = ph3.enter_context(
                    tc.tile_pool(name=f"f1ps{fh}", bufs=4, space="PSUM")
                )
                f2_ps = ph3.enter_context(
                    tc.tile_pool(name=f"f2ps{fh}", bufs=4, space="PSUM")
                )
                for sp in range(NSP3):
                    l2s = l2s_pool.tile([P, EC, SP3], f32, tag="l2s")
                    nc.sync.dma_start(
                        l2s, ln2T_d.ap()[:, :, sp * SP3 : (sp + 1) * SP3]
                    )
                    hT_sp = hT_pool.tile([P, FCH, SP3], f32, tag="hT")
                    for fc in range(FCH):
                        fps = f1_ps.tile([P, SP3], f32, tag="f1")
                        for ec in range(EC):
                            nc.tensor.matmul(
                                fps,
                                w1h[:, ec, fc * P : (fc + 1) * P],
                                l2s[:, ec, :],
                                start=(ec == 0),
                                stop=(ec == EC - 1),
                            )
                        nc.scalar.activation(
                            hT_sp[:, fc, :],
                            fps,
                            AF.Gelu,
                            bias=b1_sb[:, fh * FCH + fc : fh * FCH + fc + 1],
                        )
                    for tci in range(SP3 // P):
                        t0 = sp * SP3 + tci * P
                        if fh == 0:
                            o2c = o_pool.tile([P, E], f32, tag="o2c")
                        else:
                            o2p = o_pool.tile([P, E], f32, tag="o2p")
                            nc.sync.dma_start(o2p, o2p_d.ap()[t0 : t0 + P, :])
                            a1c2 = o_pool.tile([P, E], f32, tag="a1c2")
                            nc.sync.dma_start(a1c2, add1_d.ap()[t0 : t0 + P, :])
                            outc = o_pool.tile([P, E], f32, tag="outc")
                        for half in range(2):
                            ops2 = f2_ps.tile([P, 512], f32, tag="f2")
                            for fc in range(FCH):
                                nc.tensor.matmul(
                                    ops2,
                                    hT_sp[:, fc, tci * P : (tci + 1) * P],
                                    w2h[:, fc, half * 512 : (half + 1) * 512],
                                    start=(fc == 0),
                                    stop=(fc == FCH - 1),
                                )
                            if fh == 0:
                                nc.vector.tensor_copy(
                                    o2c[:, half * 512 : (half + 1) * 512], ops2
                                )
                            else:
                                st = os_pool.tile([P, 512], f32, tag="sum2")
                                nc.vector.tensor_tensor(
                                    st, ops2, o2p[:, half * 512 : (half + 1) * 512], OP.add
                                )
                                gt = os_pool.tile([P, 512], f32, tag="gt")
                                nc.scalar.activation(gt, st, AF.Gelu)
                                nc.vector.tensor_tensor(
                                    outc[:, half * 512 : (half + 1) * 512],
                                    gt,
                                    a1c2[:, half * 512 : (half + 1) * 512],
                                    OP.add,
                                )
                        if fh == 0:
                            nc.sync.dma_start(o2p_d.ap()[t0 : t0 + P, :], o2c)
                        else:
                            nc.sync.dma_start(y_d.ap()[t0 : t0 + P, :], outc)

    _legalize_sync_waits(nc)
    return nc


def _prep_weights(gamma1, beta1, wq, wk, wv, wo, w1, b1, w2, b2):
    """Fold LN affine params into the downstream weights (exact in fp64)."""
    g = gamma1.astype(np.float64)
    b = beta1.astype(np.float64)
    out = {}
    for name, w in (("wq", wq), ("wk", wk), ("wv", wv)):
        w2d = w.reshape(E, E).astype(np.float64)
        out[name] = (g[:, None] * w2d).astype(np.float32)
        bias = b @ w2d
        assert np.abs(bias).max() == 0.0, "nonzero LN beta needs bias path"
    out["wo"] = wo.reshape(E, E).astype(np.float32)
    w1_64 = w1.astype(np.float64)
    out["w1"] = (g[:, None] * w1_64).astype(np.float32)
    out["b1"] = (b1.astype(np.float64) + b @ w1_64).astype(np.float32)
    out["w2"] = w2.astype(np.float32)
    assert np.abs(b2).max() == 0.0, "nonzero b2 needs bias path"
    return out


def kernel(x, gamma1, beta1, wq, wk, wv, wo, w1, b1, w2, b2):
    from concourse.bass_utils import run_bass_kernel_spmd

    x = np.asarray(x, dtype=np.float32)
    B = x.shape[0]
    w = _prep_weights(
        np.asarray(gamma1), np.asarray(beta1), np.asarray(wq), np.asarray(wk),
        np.asarray(wv), np.asarray(wo), np.asarray(w1), np.asarray(b1),
        np.asarray(w2), np.asarray(b2),
    )
    ident = np.eye(P, dtype=np.float32)
    nc = build_program(T=x.shape[1], n_cores=B)
    in_maps = [
        {"x": x[b], "ident": ident, **w}
        for b in range(B)
    ]
    res = run_bass_kernel_spmd(nc, in_maps, list(range(B)))
    return np.stack([res.results[b]["y"] for b in range(B)], axis=0)
